# revision 17
# baseline (speedup 1.0000x reference)
"""Trainium2 Bass kernel for a 6-layer transformer decoder (self+cross attention).

Strategy: data-parallel over batch across 8 NeuronCores. Each core runs the
full decoder on its 8-batch-item shard, with activations kept transposed
[C, B_local*T] in SBUF so every projection is a natural lhsT.T @ rhs matmul
with a 512-wide moving dim. Matmul inputs are bf16 (fp32 PSUM accumulate);
residual stream and layernorm statistics stay fp32.

Scheduling (v2): attention weights double-buffered and prefetched a layer
ahead; filler projections emitted BEFORE the softmax colsum so the PE queue
never head-of-line blocks on the ACT exp chain; the next layer's Q
projection (half 0) is hoisted across the layer boundary to cover the final
layernorm's DVE/ACT critical path; startup DMAs ordered by first use.
"""

import numpy as np
import ml_dtypes

L, H, C, DH, FF = 6, 8, 512, 64, 2048
B, T = 64, 128
EPS = 1e-5
NCORES = 8
BL = B // NCORES          # batch items per core
TB = BL * T               # 1024 activation columns per core
NC = C // 128             # 4 channel chunks
NF = FF // 128            # 16 ff chunks
NPAIR = H // 2            # head pairs
HD = H * DH               # 512
P = 128
NORM = 1.0 / (T * C)      # layernorm 1/N, folded into the stats matmul

_BF = ml_dtypes.bfloat16

_cache = {}


def _build(n_layers=L):
    from contextlib import ExitStack

    import concourse.bass as bass  # noqa: F401
    import concourse.tile as tile
    import concourse.mybir as mybir
    from concourse import bacc

    dt = mybir.dt
    AF = mybir.ActivationFunctionType
    OP = mybir.AluOpType
    f32, bf16 = dt.float32, dt.bfloat16

    nc = bacc.Bacc("TRN2", target_bir_lowering=False, debug=False, num_devices=NCORES)

    d_xT = nc.dram_tensor("xT", [C, TB], f32, kind="ExternalInput").ap()
    d_xTb = nc.dram_tensor("xTb", [C, TB], bf16, kind="ExternalInput").ap()
    d_eT = nc.dram_tensor("eT", [C, TB], bf16, kind="ExternalInput").ap()
    d_w = {}
    for name in ("wq", "wk", "wv", "cq", "ck", "cv"):
        d_w[name] = nc.dram_tensor(name, [L, P, NC, HD], bf16, kind="ExternalInput").ap()
    d_w1 = nc.dram_tensor("w1", [L, P, NC, FF], bf16, kind="ExternalInput").ap()
    d_w2 = nc.dram_tensor("w2", [L, P, NF, C], bf16, kind="ExternalInput").ap()
    d_bqk = nc.dram_tensor("bqk", [P, L * 16], f32, kind="ExternalInput").ap()
    d_bvb = nc.dram_tensor("bvb", [L, 2, P, HD], f32, kind="ExternalInput").ap()
    d_b1 = nc.dram_tensor("b1", [P, L * NF], f32, kind="ExternalInput").ap()
    d_b2 = nc.dram_tensor("b2", [P, L * NC], f32, kind="ExternalInput").ap()
    d_out = nc.dram_tensor("outT", [C, TB], f32, kind="ExternalOutput").ap()

    with tile.TileContext(nc) as tc:
        with ExitStack() as ctx:
            cpool = ctx.enter_context(tc.tile_pool(name="const", bufs=1))
            apool = ctx.enter_context(tc.tile_pool(name="acts", bufs=1))
            wpool = ctx.enter_context(tc.tile_pool(name="wts", bufs=1))
            tpool = ctx.enter_context(tc.tile_pool(name="tmp", bufs=2))
            ps_pj = ctx.enter_context(tc.tile_pool(name="pj", bufs=3, space="PSUM"))
            ps_sc = ctx.enter_context(tc.tile_pool(name="sc", bufs=3, space="PSUM"))
            ps_ao = ctx.enter_context(tc.tile_pool(name="ao", bufs=2, space="PSUM"))

            # ---- constants (engine memsets; no DMA) ----
            ones128b = cpool.tile([P, P], bf16, tag="ones128b")
            nc.vector.memset(ones128b, 1.0)
            onesN = cpool.tile([P, P], f32, tag="onesN")
            nc.vector.memset(onesN, NORM)      # ones/65536 for LN stats matmul
            eps_t = cpool.tile([P, 1], f32, tag="eps")
            nc.vector.memset(eps_t, EPS)
            zero_t = cpool.tile([P, 1], f32, tag="zero")
            nc.vector.memset(zero_t, 0.0)

            # ---- persistent activations (kc-major merged tiles per half) ----
            xres = [apool.tile([P, NC * 512], f32, tag=f"xres{h}", name=f"xres{h}")
                    for h in range(2)]
            xn = [apool.tile([P, NC * 512], bf16, tag=f"xn{h}", name=f"xn{h}")
                  for h in range(2)]
            eTs = [apool.tile([P, NC * 512], bf16, tag=f"eT{h}", name=f"eT{h}")
                   for h in range(2)]

            def dma_act(dst_list, src, h, dtype_slice=True):
                for k in range(NC):
                    rs = slice(k * P, (k + 1) * P)
                    cs_ = slice(h * 512, (h + 1) * 512)
                    ts_ = slice(k * 512, (k + 1) * 512)
                    nc.sync.dma_start(out=dst_list[h][:, ts_], in_=src[rs, cs_])

            # startup DMAs ordered by first use: xn (first projections) first,
            # then SA weights, encoder half 0 + ck (SA-h0 fillers), residual
            # half 0 (SA-h0 attn-out evict), then the rest.
            dma_act(xn, d_xTb, 0)
            dma_act(xn, d_xTb, 1)
            bqk_s = cpool.tile([P, L * 16], f32, tag="bqk")
            nc.sync.dma_start(out=bqk_s, in_=d_bqk)

            # warm-up during the input-DMA window: ~4us of dummy matmuls on
            # the const tile flips the PE HAM clock-gate to 8/8 before the
            # first real projection, and a dummy exp pulls the one ACT
            # table load off the critical path.
            wu_t = tpool.tile([P, 1], f32, tag="wu", name="wu")
            nc.scalar.activation(wu_t, eps_t, AF.Exp, bias=zero_t, scale=1.0)
            for wi in range(40):
                wup = ps_pj.tile([P, P], f32, tag="pj", name="pj")
                nc.tensor.matmul(wup, ones128b, ones128b, start=True, stop=True)

            W_ATT = ("wq", "wk", "wv", "cq", "ck", "cv")

            def load_attn_w(l, names=W_ATT):
                wts = {}
                for name in names:
                    w = wpool.tile([P, NC, HD], bf16, tag=name, name=name, bufs=2)
                    nc.sync.dma_start(out=w, in_=d_w[name][l])
                    wts[name] = w
                return wts

            def load_bv(l):
                bvs = wpool.tile([P, HD], f32, tag="bvs", name="bvs", bufs=2)
                nc.sync.dma_start(out=bvs, in_=d_bvb[l, 0])
                bvc = wpool.tile([P, HD], f32, tag="bvc", name="bvc", bufs=2)
                nc.sync.dma_start(out=bvc, in_=d_bvb[l, 1])
                return bvs, bvc

            def load_ffn_w(l):
                w1s = wpool.tile([P, NC, FF], bf16, tag="w1", name="w1")
                nc.sync.dma_start(out=w1s, in_=d_w1[l])
                w2s = wpool.tile([P, NF, C], bf16, tag="w2", name="w2")
                nc.sync.dma_start(out=w2s, in_=d_w2[l])
                return w1s, w2s

            wts = load_attn_w(0, names=("wq", "wk", "wv"))
            dma_act(eTs, d_eT, 0)
            wts.update(load_attn_w(0, names=("ck",)))
            dma_act(xres, d_xT, 0)
            wts.update(load_attn_w(0, names=("cv",)))
            bvs, bvc = load_bv(0)
            wts.update(load_attn_w(0, names=("cq",)))
            dma_act(eTs, d_eT, 1)
            dma_act(xres, d_xT, 1)
            w1s, w2s = load_ffn_w(0)
            b1_s = cpool.tile([P, L * NF], f32, tag="b1")
            nc.sync.dma_start(out=b1_s, in_=d_b1)
            b2_s = cpool.tile([P, L * NC], f32, tag="b2")
            nc.sync.dma_start(out=b2_s, in_=d_b2)

            def xsl(h2, kc, bb=None):
                if bb is None:
                    return slice(kc * 512, (kc + 1) * 512)
                return slice(kc * 512 + bb * P, kc * 512 + (bb + 1) * P)

            qT = [[apool.tile([P, 512], bf16, tag=f"qT{k}_{h}", name=f"qT{k}_{h}")
                   for h in range(2)] for k in range(NC)]
            kT = [[apool.tile([P, 512], bf16, tag=f"kT{k}_{h}", name=f"kT{k}_{h}")
                   for h in range(2)] for k in range(NC)]
            kcT = [[apool.tile([P, 512], bf16, tag=f"kcT{k}_{h}", name=f"kcT{k}_{h}")
                    for h in range(2)] for k in range(NC)]
            vS = [apool.tile([P, HD], bf16, tag=f"v{b}", name=f"v{b}") for b in range(BL)]
            vC = [apool.tile([P, HD], bf16, tag=f"vc{b}", name=f"vc{b}") for b in range(BL)]
            hT = [apool.tile([P, 512], bf16, tag=f"hT{f}", name=f"hT{f}") for f in range(NF)]

            def proj_qk(dst, wt, src, col_of, scale, halves=(0, 1), evict="act"):
                for h2 in halves:
                    for mc in range(NC):
                        pj = ps_pj.tile([P, 512], f32, tag="pj", name="pj")
                        for kc in range(NC):
                            nc.tensor.matmul(pj, wt[:, kc, mc * P:(mc + 1) * P],
                                             src[h2][:, xsl(h2, kc)],
                                             start=(kc == 0), stop=(kc == NC - 1))
                        col = col_of(mc)
                        bias = bqk_s[:, col:col + 1]
                        if evict == "act":
                            nc.scalar.activation(dst[mc][h2], pj, AF.Identity,
                                                 bias=bias, scale=scale)
                        elif scale == 1.0:
                            # DVE evict keeps the ACT queue free for the
                            # softmax exp/recip chain inside attention
                            nc.vector.tensor_scalar(dst[mc][h2], pj, bias, None,
                                                    op0=OP.add)
                        else:
                            nc.vector.tensor_scalar(dst[mc][h2], pj, scale, bias,
                                                    op0=OP.mult, op1=OP.add)

            def proj_v(dst, wt, src, bias_tile, bs):
                for b in bs:
                    h2, bb = divmod(b, 4)
                    pj = ps_pj.tile([P, 512], f32, tag="pj", name="pj")
                    for kc in range(NC):
                        nc.tensor.matmul(pj, src[h2][:, xsl(h2, kc, bb)],
                                         wt[:, kc, :],
                                         start=(kc == 0), stop=(kc == NC - 1))
                    nc.vector.tensor_tensor(dst[b], pj, bias_tile, op=OP.add)

            def attention_half(kTl, vl, ST, h2, fillerA=None, fillerB=None):
                # Per half (4 batch items). ACT ops batched by function; PE
                # filler work emitted BEFORE the colsum matmuls so the PE
                # queue doesn't head-of-line block on the ACT exp chain.
                expTs = {}
                pending = []
                for bb in range(4):
                    expT = tpool.tile([P, TB], bf16, tag="expT", name="expT",
                                      bufs=4)
                    sce = ps_sc.tile([P, 512], f32, tag="sc", name="sc")
                    sco = ps_sc.tile([P, 512], f32, tag="sc", name="sc")
                    for p in range(NPAIR):
                        nc.tensor.matmul(sce[:, p * P:(p + 1) * P],
                                         kTl[p][h2][0:64, bb * P:(bb + 1) * P],
                                         qT[p][h2][0:64, bb * P:(bb + 1) * P],
                                         start=True, stop=True,
                                         tile_position=(0, 0))
                        nc.tensor.matmul(sco[:, p * P:(p + 1) * P],
                                         kTl[p][h2][64:128, bb * P:(bb + 1) * P],
                                         qT[p][h2][64:128, bb * P:(bb + 1) * P],
                                         start=True, stop=True,
                                         tile_position=(64, 0))
                    pending.append((bb, expT, sce, sco))
                    expTs[bb] = expT
                    if len(pending) == 2 or bb == 3:
                        for _bb, _e, _sce, _sco in pending:
                            nc.scalar.activation(_e[:, 0:512], _sce, AF.Exp,
                                                 bias=zero_t, scale=1.0)
                            nc.scalar.activation(_e[:, 512:1024], _sco, AF.Exp,
                                                 bias=zero_t, scale=1.0)
                        pending = []
                if fillerA is not None:
                    fillerA()
                # Per bb: colsum matmuls, then DVE reciprocal_approx_fast
                # (~18 correct bits; keeps ACT on one table set), then ONE
                # fused [P,1024] normalize multiply. Tight per-bb emission so
                # the latency-critical softmax chain is never queued behind
                # bulk DVE work in the engine FIFO.
                for bb in range(4):
                    ddr2 = tpool.tile([P, TB], f32, tag="ddr", name="ddr",
                                      bufs=3)
                    for j in range(2):
                        sl = slice(j * 512, (j + 1) * 512)
                        dsum = ps_sc.tile([P, 512], f32, tag="sc", name="sc")
                        nc.tensor.matmul(dsum, ones128b, expTs[bb][:, sl],
                                         start=True, stop=True)
                        nc.vector.reciprocal_approx_fast(out=ddr2[:, sl],
                                                         in_=dsum)
                    nc.vector.tensor_tensor(expTs[bb], expTs[bb], ddr2,
                                            op=OP.mult)
                if fillerB is not None:
                    fillerB()
                x3 = xres[h2].rearrange("p (k n) -> p k n", k=NC)
                for bb in range(4):
                    b = h2 * 4 + bb
                    ao = ps_ao.tile([P, 512], f32, tag="ao", name="ao")
                    for p in range(NPAIR):
                        for j in range(2):
                            h = 2 * p + j
                            pos = (h % 2) * 512 + (h // 2) * P
                            nc.tensor.matmul(ao[j * 64:(j + 1) * 64, p * P:(p + 1) * P],
                                             vl[b][:, h * 64:(h + 1) * 64],
                                             expTs[bb][:, pos:pos + P],
                                             start=True, stop=True,
                                             tile_position=(0, j * 64))
                    dst = x3[:, :, bb * P:(bb + 1) * P]
                    nc.vector.scalar_tensor_tensor(dst,
                                                   ao.rearrange("p (k n) -> p k n", k=NC),
                                                   0.0, dst,
                                                   op0=OP.add, op1=OP.add,
                                                   accum_out=ST[:, bb, 0:1])

            def ln_stats(ST, h2, nsum=1):
                # DVE part of layernorm: per-item sums were accumulated into
                # ST[:, b, 0:nsum] by the residual-evict ops; a fused
                # square+reduce per item fills ST[:, b, 4].
                x3 = xres[h2].rearrange("p (k n) -> p k n", k=NC)
                for bb in range(4):
                    sq = tpool.tile([P, 512], bf16, tag="sq", name="sq")
                    src = x3[:, :, bb * P:(bb + 1) * P]
                    nc.vector.scalar_tensor_tensor(
                        sq.rearrange("p (k n) -> p k n", k=NC), src, 1.0, src,
                        op0=OP.mult, op1=OP.mult,
                        accum_out=ST[:, bb, 4:5])
                if nsum == 1:
                    # cols 1-3 unwritten in attention mode; zero them so the
                    # stats matmul never reads uninitialized sbuf
                    nc.vector.memset(ST[:, :, 1:4], 0.0)

            def ln_apply(ST, h2, last=False, nsum=1):
                # PE partition-reduce + normalize. 1/sqrt(var+eps) via the
                # quake bit-trick + 2 Newton steps, all on DVE — the ACT
                # engine only ever sees Exp/Identity/Relu/Square, so a single
                # activation table set serves the whole kernel (no reloads).
                x3 = xres[h2].rearrange("p (k n) -> p k n", k=NC)
                tot = ps_pj.tile([P, 20], f32, tag="pj", name="pj")
                nc.tensor.matmul(tot, onesN,
                                 ST.rearrange("p a b -> p (a b)"),
                                 start=True, stop=True)
                tot3 = tot.rearrange("p (a b) -> p a b", b=5)
                mm_ = tpool.tile([P, 4], f32, tag="mm_", name="mm_")
                if nsum == 1:
                    nc.vector.tensor_copy(out=mm_, in_=tot3[:, :, 0])
                else:
                    nc.vector.reduce_sum(mm_, tot3[:, :, 0:nsum],
                                         axis=mybir.AxisListType.X)
                var = tpool.tile([P, 4], f32, tag="var", name="var")
                nc.vector.tensor_tensor(var, mm_, mm_, op=OP.mult)
                # var = (sumsq + eps) - mean^2
                nc.vector.scalar_tensor_tensor(var, tot3[:, :, 4], EPS, var,
                                               op0=OP.add, op1=OP.subtract)
                rsi = tpool.tile([P, 4], mybir.dt.int32, tag="rsi", name="rsi")
                nc.vector.tensor_scalar(rsi, var.bitcast(mybir.dt.int32), 1,
                                        None, op0=OP.logical_shift_right)
                # K - x computed as ~x + (K+1); walrus refuses mixed
                # bitwise/arith op pairs in one instruction, so split
                nc.vector.tensor_scalar(rsi, rsi, -1, None, op0=OP.bitwise_xor)
                nc.vector.tensor_scalar(rsi, rsi, 0x5f3759e0, None, op0=OP.add)
                r0 = rsi.bitcast(f32)
                rt = tpool.tile([P, 4], f32, tag="rt", name="rt")
                ru = tpool.tile([P, 4], f32, tag="ru", name="ru")
                rr = tpool.tile([P, 4], f32, tag="rr", name="rr")
                nc.vector.tensor_tensor(rt, var, r0, op=OP.mult)
                nc.vector.tensor_tensor(rt, rt, r0, op=OP.mult)
                nc.vector.tensor_scalar(ru, rt, -0.5, 1.5, op0=OP.mult, op1=OP.add)
                nc.vector.tensor_tensor(rr, r0, ru, op=OP.mult)
                nc.vector.tensor_tensor(rt, var, rr, op=OP.mult)
                nc.vector.tensor_tensor(rt, rt, rr, op=OP.mult)
                nc.vector.tensor_scalar(ru, rt, -0.5, 1.5, op0=OP.mult, op1=OP.add)
                nc.vector.tensor_tensor(rr, rr, ru, op=OP.mult)
                # nm = -mean*rr so the fp32 xres update can run on ACT as
                # Identity(x*rr + nm), offloading DVE
                nm = tpool.tile([P, 4], f32, tag="nm", name="nm")
                nc.vector.tensor_tensor(nm, mm_, rr, op=OP.mult)
                nc.vector.tensor_scalar(nm, nm, -1.0, None, op0=OP.mult)
                # both normalized copies run on ACT (Identity with per-item
                # scale/bias APs), keeping the DVE free for the softmax chain
                xn3 = xn[h2].rearrange("p (k n) -> p k n", k=NC)
                for bb in range(4):
                    src = x3[:, :, bb * P:(bb + 1) * P]
                    if not last:
                        nc.scalar.activation(xn3[:, :, bb * P:(bb + 1) * P],
                                             src, AF.Identity,
                                             bias=nm[:, bb:bb + 1],
                                             scale=rr[:, bb:bb + 1])
                for bb in range(4):
                    src = x3[:, :, bb * P:(bb + 1) * P]
                    nc.scalar.activation(src, src, AF.Identity,
                                         bias=nm[:, bb:bb + 1],
                                         scale=rr[:, bb:bb + 1])

            def out_dma(h):
                for k in range(NC):
                    nc.sync.dma_start(
                        out=d_out[k * P:(k + 1) * P, h * 512:(h + 1) * 512],
                        in_=xres[h][:, k * 512:(k + 1) * 512])

            for l in range(n_layers):
                wts_next = load_attn_w(l + 1) if l + 1 < n_layers else None
                if wts_next is not None:
                    bvs_n, bvc_n = load_bv(l + 1)

                def new_st():
                    return tpool.tile([P, 4, 5], f32, tag="ST", name="ST", bufs=8)

                qcol = lambda mc, ml=l: (ml * 4 + 0) * 4 + mc
                kcol = lambda mc, ml=l: (ml * 4 + 1) * 4 + mc
                cqcol = lambda mc, ml=l: (ml * 4 + 2) * 4 + mc
                ckcol = lambda mc, ml=l: (ml * 4 + 3) * 4 + mc

                # --- QKV projections (wq/wk half 0 hoisted into layer l-1) ---
                ST1 = [new_st(), new_st()]
                if l == 0:
                    proj_qk(qT, wts["wq"], xn, qcol, 0.125)
                    proj_qk(kT, wts["wk"], xn, kcol, 1.0)
                    proj_v(vS, wts["wv"], xn, bvs, range(BL))
                else:
                    proj_v(vS, wts["wv"], xn, bvs, range(0, 4))
                    proj_qk(qT, wts["wq"], xn, qcol, 0.125, halves=(1,))
                    proj_qk(kT, wts["wk"], xn, kcol, 1.0)
                    proj_v(vS, wts["wv"], xn, bvs, range(4, 8))

                # --- self attention (cross K/V emitted as PE filler) ---
                def fA_sa0():
                    proj_qk(kcT, wts["ck"], eTs, ckcol, 1.0, halves=(0,))

                def fB_sa0():
                    proj_v(vC, wts["cv"], eTs, bvc, range(0, 4))

                def fA_sa1():
                    proj_qk(kcT, wts["ck"], eTs, ckcol, 1.0, halves=(1,))
                    # LN1-h0 apply interleaved: its stats matmul lands after
                    # ~5us of SA-h1 PE work so the DVE squares chain is done
                    ln_apply(ST1[0], 0)

                def fB_sa1():
                    proj_v(vC, wts["cv"], eTs, bvc, range(4, 8))

                attention_half(kT, vS, ST1[0], 0, fillerA=fA_sa0, fillerB=fB_sa0)
                ln_stats(ST1[0], 0)
                attention_half(kT, vS, ST1[1], 1, fillerA=fA_sa1, fillerB=fB_sa1)
                ln_stats(ST1[1], 1)
                proj_qk(qT, wts["cq"], xn, cqcol, 0.125, halves=(0,))
                ln_apply(ST1[1], 1)

                # --- cross attention ---
                ST2 = [new_st(), new_st()]

                def fA_ca0():
                    proj_qk(qT, wts["cq"], xn, cqcol, 0.125, halves=(1,))

                def ffn_w1(h2, fcs):
                    for fc in fcs:
                        pj = ps_pj.tile([P, 512], f32, tag="pj", name="pj")
                        for kc in range(NC):
                            nc.tensor.matmul(pj, w1s[:, kc, fc * P:(fc + 1) * P],
                                             xn[h2][:, xsl(h2, kc)],
                                             start=(kc == 0), stop=(kc == NC - 1))
                        col = l * NF + fc
                        nc.scalar.activation(hT[fc], pj, AF.Relu,
                                             bias=b1_s[:, col:col + 1], scale=1.0)

                def fA_ca1():
                    # LN2-h0 must be applied before its consumer ffn_w1(0)
                    ln_apply(ST2[0], 0)
                    ffn_w1(0, range(0, 3))

                def fB_ca1():
                    ffn_w1(0, range(3, 8))

                attention_half(kcT, vC, ST2[0], 0, fillerA=fA_ca0)
                ln_stats(ST2[0], 0)
                attention_half(kcT, vC, ST2[1], 1, fillerA=fA_ca1, fillerB=fB_ca1)
                ln_stats(ST2[1], 1)
                ffn_w1(0, range(8, NF))
                ln_apply(ST2[1], 1)

                # --- feed-forward ---
                ST3 = [new_st(), new_st()]

                def ffn_w2(h2, ST):
                    for mc in range(NC):
                        pj = ps_pj.tile([P, 512], f32, tag="pj", name="pj")
                        for fc in range(NF):
                            nc.tensor.matmul(pj, w2s[:, fc, mc * P:(mc + 1) * P],
                                             hT[fc],
                                             start=(fc == 0), stop=(fc == NF - 1))
                        b2col = b2_s[:, l * NC + mc:l * NC + mc + 1]
                        for bb in range(4):
                            dst = xres[h2][:, xsl(h2, mc, bb)]
                            nc.vector.scalar_tensor_tensor(dst,
                                                           pj[:, bb * P:(bb + 1) * P],
                                                           b2col, dst,
                                                           op0=OP.add, op1=OP.add,
                                                           accum_out=ST[:, bb, mc:mc + 1])

                last = l == n_layers - 1
                ffn_w2(0, ST3[0])
                ln_stats(ST3[0], 0)
                ffn_w1(1, range(0, 4))
                ln_apply(ST3[0], 0, last=last)
                if last:
                    out_dma(0)
                ffn_w1(1, range(4, NF))
                if l + 1 < n_layers:
                    w1s_n, w2s_n = load_ffn_w(l + 1)
                ffn_w2(1, ST3[1])
                ln_stats(ST3[1], 1)
                if wts_next is not None:
                    # hoist: next layer's Q projection (half 0) keeps the PE
                    # busy while the final layernorm's DVE/ACT chain runs
                    proj_qk(qT, wts_next["wq"], xn,
                            lambda mc, ml=l + 1: (ml * 4 + 0) * 4 + mc,
                            0.125, halves=(0,))
                ln_apply(ST3[1], 1, last=last)
                if last:
                    out_dma(1)
                else:
                    wts = wts_next
                    bvs, bvc = bvs_n, bvc_n
                    w1s, w2s = w1s_n, w2s_n

    nc.compile()
    return nc


def _prep_shared(inputs):
    """Host-side weight repacking (shared across cores)."""
    def packw(w):  # [L,H,C,DH] -> [L,128,NC,H*DH]  (c = kc*128+p)
        w2 = np.ascontiguousarray(w.transpose(0, 2, 1, 3)).reshape(L, C, HD)
        return np.ascontiguousarray(
            w2.reshape(L, NC, P, HD).transpose(0, 2, 1, 3)).astype(_BF)

    shared = {}
    for nm, key in (("wq", "sa_wq"), ("wk", "sa_wk"), ("wv", "sa_wv"),
                    ("cq", "ca_wq"), ("ck", "ca_wk"), ("cv", "ca_wv")):
        shared[nm] = packw(inputs[key])
    shared["w1"] = np.ascontiguousarray(
        inputs["ff_w1"].reshape(L, NC, P, FF).transpose(0, 2, 1, 3)).astype(_BF)
    shared["w2"] = np.ascontiguousarray(
        inputs["ff_w2"].reshape(L, NF, P, C).transpose(0, 2, 1, 3)).astype(_BF)

    bqk = np.zeros((P, L * 16), np.float32)
    for l in range(L):
        for mi, (bias, s) in enumerate((
                (inputs["sa_bq"][l], 0.125), (inputs["sa_bk"][l], 1.0),
                (inputs["ca_bq"][l], 0.125), (inputs["ca_bk"][l], 1.0))):
            flat = bias.reshape(HD).astype(np.float32) * s
            for mc in range(NC):
                bqk[:, (l * 4 + mi) * 4 + mc] = flat[mc * P:(mc + 1) * P]
    shared["bqk"] = bqk

    bv = np.stack([inputs["sa_bv"].reshape(L, HD),
                   inputs["ca_bv"].reshape(L, HD)], axis=1).astype(np.float32)
    shared["bvb"] = np.ascontiguousarray(
        np.broadcast_to(bv[:, :, None, :], (L, 2, P, HD)))

    b1 = np.zeros((P, L * NF), np.float32)
    for l in range(L):
        for fc in range(NF):
            b1[:, l * NF + fc] = inputs["ff_b1"][l, fc * P:(fc + 1) * P]
    shared["b1"] = b1
    b2 = np.zeros((P, L * NC), np.float32)
    for l in range(L):
        for mc in range(NC):
            b2[:, l * NC + mc] = inputs["ff_b2"][l, mc * P:(mc + 1) * P]
    shared["b2"] = b2
    return shared


LAST_RESULT = None


def _install_ntff_hook():
    """Register the axon NTFF profile hook that the image's antenv lacks.

    Only used for local benchmarking (KERNEL_TRACE=1); inert otherwise.
    """
    import sys
    import types
    try:
        import antenv
        if getattr(antenv, "axon_hooks", None) is not None:
            return
        from trn_agent_boot.trn_boot import _ntff_profile_via_ctypes
        mod = types.ModuleType("antenv.axon_hooks")
        mod._hook = _ntff_profile_via_ctypes("/opt/axon/libaxon_pjrt.so")

        def get_axon_ntff_profile_hook():
            return mod._hook

        def set_axon_ntff_profile_hook(h):
            mod._hook = h

        mod.get_axon_ntff_profile_hook = get_axon_ntff_profile_hook
        mod.set_axon_ntff_profile_hook = set_axon_ntff_profile_hook
        sys.modules["antenv.axon_hooks"] = mod
        antenv.axon_hooks = mod
    except Exception as e:  # pragma: no cover - profiling is best-effort
        print(f"ntff hook install failed: {e}")


def kernel(**inputs):
    global LAST_RESULT
    import os
    inputs = {k: np.asarray(v) for k, v in inputs.items()}
    if "nc" not in _cache:
        _cache["nc"] = _build()
    nc = _cache["nc"]

    shared = _prep_shared(inputs)
    x = inputs["x"].astype(np.float32)
    enc = inputs["encoder_output"].astype(np.float32)

    in_maps = []
    for core in range(NCORES):
        sl = slice(core * BL, (core + 1) * BL)
        xT = np.ascontiguousarray(x[sl].transpose(2, 0, 1)).reshape(C, TB)
        eT = np.ascontiguousarray(enc[sl].transpose(2, 0, 1)).reshape(C, TB)
        m = dict(shared)
        m["xT"] = xT
        m["xTb"] = xT.astype(_BF)
        m["eT"] = eT.astype(_BF)
        in_maps.append(m)

    trace = bool(int(os.environ.get("KERNEL_TRACE", "0")))
    if trace:
        _install_ntff_hook()
    from concourse.bass_utils import run_bass_kernel_spmd
    res = run_bass_kernel_spmd(nc, in_maps, list(range(NCORES)), trace=trace,
                               trace_cores=[0])
    LAST_RESULT = res

    out = np.empty((B, T, C), np.float32)
    for core in range(NCORES):
        outT = res.results[core]["outT"]  # [C, TB]
        out[core * BL:(core + 1) * BL] = outT.reshape(C, BL, T).transpose(1, 2, 0)
    return out


# revision 20
# speedup vs baseline: 1.0504x; 1.0504x over previous
"""Trainium2 Bass kernel for a 6-layer transformer decoder (self+cross attention).

Strategy: data-parallel over batch across 8 NeuronCores. Each core runs the
full decoder on its 8-batch-item shard, with activations kept transposed
[C, B_local*T] in SBUF so every projection is a natural lhsT.T @ rhs matmul
with a 512-wide moving dim. Matmul inputs are bf16 (fp32 PSUM accumulate);
residual stream and layernorm statistics stay fp32.

Scheduling (v2): attention weights double-buffered and prefetched a layer
ahead; filler projections emitted BEFORE the softmax colsum so the PE queue
never head-of-line blocks on the ACT exp chain; the next layer's Q
projection (half 0) is hoisted across the layer boundary to cover the final
layernorm's DVE/ACT critical path; startup DMAs ordered by first use.
"""

import numpy as np
import ml_dtypes

L, H, C, DH, FF = 6, 8, 512, 64, 2048
B, T = 64, 128
EPS = 1e-5
NCORES = 8
BL = B // NCORES          # batch items per core
TB = BL * T               # 1024 activation columns per core
NC = C // 128             # 4 channel chunks
NF = FF // 128            # 16 ff chunks
NPAIR = H // 2            # head pairs
HD = H * DH               # 512
P = 128
NORM = 1.0 / (T * C)      # layernorm 1/N, folded into the stats matmul

_BF = ml_dtypes.bfloat16

_cache = {}


def _build(n_layers=L):
    from contextlib import ExitStack

    import concourse.bass as bass  # noqa: F401
    import concourse.tile as tile
    import concourse.mybir as mybir
    from concourse import bacc

    dt = mybir.dt
    AF = mybir.ActivationFunctionType
    OP = mybir.AluOpType
    f32, bf16 = dt.float32, dt.bfloat16

    nc = bacc.Bacc("TRN2", target_bir_lowering=False, debug=False, num_devices=NCORES)

    d_xT = nc.dram_tensor("xT", [C, TB], f32, kind="ExternalInput").ap()
    d_xTb = nc.dram_tensor("xTb", [C, TB], bf16, kind="ExternalInput").ap()
    d_eT = nc.dram_tensor("eT", [C, TB], bf16, kind="ExternalInput").ap()
    d_w = {}
    for name in ("wq", "wk", "wv", "cq", "ck", "cv"):
        d_w[name] = nc.dram_tensor(name, [L, P, NC, HD], bf16, kind="ExternalInput").ap()
    d_w1 = nc.dram_tensor("w1", [L, P, NC, FF], bf16, kind="ExternalInput").ap()
    d_w2 = nc.dram_tensor("w2", [L, P, NF, C], bf16, kind="ExternalInput").ap()
    d_bqk = nc.dram_tensor("bqk", [P, L * 16], f32, kind="ExternalInput").ap()
    d_bvb = nc.dram_tensor("bvb", [L, 2, P, HD], f32, kind="ExternalInput").ap()
    d_b1 = nc.dram_tensor("b1", [P, L * NF], f32, kind="ExternalInput").ap()
    d_b2 = nc.dram_tensor("b2", [P, L * NC], f32, kind="ExternalInput").ap()
    d_out = nc.dram_tensor("outT", [C, TB], f32, kind="ExternalOutput").ap()

    with tile.TileContext(nc) as tc:
        with ExitStack() as ctx:
            cpool = ctx.enter_context(tc.tile_pool(name="const", bufs=1))
            apool = ctx.enter_context(tc.tile_pool(name="acts", bufs=1))
            wpool = ctx.enter_context(tc.tile_pool(name="wts", bufs=1))
            tpool = ctx.enter_context(tc.tile_pool(name="tmp", bufs=2))
            ps_pj = ctx.enter_context(tc.tile_pool(name="pj", bufs=3, space="PSUM"))
            ps_sc = ctx.enter_context(tc.tile_pool(name="sc", bufs=3, space="PSUM"))
            ps_ao = ctx.enter_context(tc.tile_pool(name="ao", bufs=2, space="PSUM"))

            # ---- constants (engine memsets; no DMA) ----
            ones128b = cpool.tile([P, P], bf16, tag="ones128b")
            nc.vector.memset(ones128b, 1.0)
            onesN = cpool.tile([P, P], f32, tag="onesN")
            nc.vector.memset(onesN, NORM)      # ones/65536 for LN stats matmul
            eps_t = cpool.tile([P, 1], f32, tag="eps")
            nc.vector.memset(eps_t, EPS)
            zero_t = cpool.tile([P, 1], f32, tag="zero")
            nc.vector.memset(zero_t, 0.0)

            # ---- persistent activations (kc-major merged tiles per half) ----
            xres = [apool.tile([P, NC * 512], f32, tag=f"xres{h}", name=f"xres{h}")
                    for h in range(2)]
            xn = [apool.tile([P, NC * 512], bf16, tag=f"xn{h}", name=f"xn{h}")
                  for h in range(2)]
            eTs = [apool.tile([P, NC * 512], bf16, tag=f"eT{h}", name=f"eT{h}")
                   for h in range(2)]

            def dma_act(dst_list, src, h, dtype_slice=True):
                for k in range(NC):
                    rs = slice(k * P, (k + 1) * P)
                    cs_ = slice(h * 512, (h + 1) * 512)
                    ts_ = slice(k * 512, (k + 1) * 512)
                    nc.sync.dma_start(out=dst_list[h][:, ts_], in_=src[rs, cs_])

            # startup DMAs ordered by first use: xn (first projections) first,
            # then SA weights, encoder half 0 + ck (SA-h0 fillers), residual
            # half 0 (SA-h0 attn-out evict), then the rest.
            dma_act(xn, d_xTb, 0)
            dma_act(xn, d_xTb, 1)
            bqk_s = cpool.tile([P, L * 16], f32, tag="bqk")
            nc.sync.dma_start(out=bqk_s, in_=d_bqk)

            # warm-up during the input-DMA window: ~4us of dummy matmuls on
            # the const tile flips the PE HAM clock-gate to 8/8 before the
            # first real projection, and a dummy exp pulls the one ACT
            # table load off the critical path.
            wu_t = tpool.tile([P, 1], f32, tag="wu", name="wu")
            nc.scalar.activation(wu_t, eps_t, AF.Exp, bias=zero_t, scale=1.0)
            for wi in range(40):
                wup = ps_pj.tile([P, P], f32, tag="pj", name="pj")
                nc.tensor.matmul(wup, ones128b, ones128b, start=True, stop=True)

            W_ATT = ("wq", "wk", "wv", "cq", "ck", "cv")

            def load_attn_w(l, names=W_ATT):
                wts = {}
                for name in names:
                    w = wpool.tile([P, NC, HD], bf16, tag=name, name=name, bufs=2)
                    nc.sync.dma_start(out=w, in_=d_w[name][l])
                    wts[name] = w
                return wts

            def load_bv(l):
                bvs = wpool.tile([P, HD], f32, tag="bvs", name="bvs", bufs=2)
                nc.sync.dma_start(out=bvs, in_=d_bvb[l, 0])
                bvc = wpool.tile([P, HD], f32, tag="bvc", name="bvc", bufs=2)
                nc.sync.dma_start(out=bvc, in_=d_bvb[l, 1])
                return bvs, bvc

            def load_ffn_w(l):
                w1s = wpool.tile([P, NC, FF], bf16, tag="w1", name="w1")
                nc.sync.dma_start(out=w1s, in_=d_w1[l])
                w2s = wpool.tile([P, NF, C], bf16, tag="w2", name="w2")
                nc.sync.dma_start(out=w2s, in_=d_w2[l])
                return w1s, w2s

            wts = load_attn_w(0, names=("wq", "wk", "wv"))
            dma_act(eTs, d_eT, 0)
            wts.update(load_attn_w(0, names=("ck",)))
            dma_act(xres, d_xT, 0)
            wts.update(load_attn_w(0, names=("cv",)))
            bvs, bvc = load_bv(0)
            wts.update(load_attn_w(0, names=("cq",)))
            dma_act(eTs, d_eT, 1)
            dma_act(xres, d_xT, 1)
            w1s, w2s = load_ffn_w(0)
            b1_s = cpool.tile([P, L * NF], f32, tag="b1")
            nc.sync.dma_start(out=b1_s, in_=d_b1)
            b2_s = cpool.tile([P, L * NC], f32, tag="b2")
            nc.sync.dma_start(out=b2_s, in_=d_b2)

            def xsl(h2, kc, bb=None):
                if bb is None:
                    return slice(kc * 512, (kc + 1) * 512)
                return slice(kc * 512 + bb * P, kc * 512 + (bb + 1) * P)

            qT = [[apool.tile([P, 512], bf16, tag=f"qT{k}_{h}", name=f"qT{k}_{h}")
                   for h in range(2)] for k in range(NC)]
            kT = [[apool.tile([P, 512], bf16, tag=f"kT{k}_{h}", name=f"kT{k}_{h}")
                   for h in range(2)] for k in range(NC)]
            kcT = [[apool.tile([P, 512], bf16, tag=f"kcT{k}_{h}", name=f"kcT{k}_{h}")
                    for h in range(2)] for k in range(NC)]
            vS = [apool.tile([P, HD], bf16, tag=f"v{b}", name=f"v{b}") for b in range(BL)]
            vC = [apool.tile([P, HD], bf16, tag=f"vc{b}", name=f"vc{b}") for b in range(BL)]
            hT = [apool.tile([P, 512], bf16, tag=f"hT{f}", name=f"hT{f}") for f in range(NF)]

            def proj_qk(dst, wt, src, col_of, scale, halves=(0, 1), evict="act"):
                for h2 in halves:
                    for mc in range(NC):
                        pj = ps_pj.tile([P, 512], f32, tag="pj", name="pj")
                        for kc in range(NC):
                            nc.tensor.matmul(pj, wt[:, kc, mc * P:(mc + 1) * P],
                                             src[h2][:, xsl(h2, kc)],
                                             start=(kc == 0), stop=(kc == NC - 1))
                        col = col_of(mc)
                        bias = bqk_s[:, col:col + 1]
                        if evict == "act":
                            nc.scalar.activation(dst[mc][h2], pj, AF.Identity,
                                                 bias=bias, scale=scale)
                        elif scale == 1.0:
                            # DVE evict keeps the ACT queue free for the
                            # softmax exp/recip chain inside attention
                            nc.vector.tensor_scalar(dst[mc][h2], pj, bias, None,
                                                    op0=OP.add)
                        else:
                            nc.vector.tensor_scalar(dst[mc][h2], pj, scale, bias,
                                                    op0=OP.mult, op1=OP.add)

            def proj_v(dst, wt, src, bias_tile, bs):
                for b in bs:
                    h2, bb = divmod(b, 4)
                    pj = ps_pj.tile([P, 512], f32, tag="pj", name="pj")
                    for kc in range(NC):
                        nc.tensor.matmul(pj, src[h2][:, xsl(h2, kc, bb)],
                                         wt[:, kc, :],
                                         start=(kc == 0), stop=(kc == NC - 1))
                    nc.vector.tensor_tensor(dst[b], pj, bias_tile, op=OP.add)

            def attention_half(kTl, vl, ST, h2, fillerA=None, fillerB=None):
                # Per half (4 batch items). ACT ops batched by function; PE
                # filler work emitted BEFORE the colsum matmuls so the PE
                # queue doesn't head-of-line block on the ACT exp chain.
                expTs = {}
                pending = []
                for bb in range(4):
                    expT = tpool.tile([P, TB], bf16, tag="expT", name="expT",
                                      bufs=4)
                    sce = ps_sc.tile([P, 512], f32, tag="sc", name="sc")
                    sco = ps_sc.tile([P, 512], f32, tag="sc", name="sc")
                    for p in range(NPAIR):
                        nc.tensor.matmul(sce[:, p * P:(p + 1) * P],
                                         kTl[p][h2][0:64, bb * P:(bb + 1) * P],
                                         qT[p][h2][0:64, bb * P:(bb + 1) * P],
                                         start=True, stop=True,
                                         tile_position=(0, 0))
                        nc.tensor.matmul(sco[:, p * P:(p + 1) * P],
                                         kTl[p][h2][64:128, bb * P:(bb + 1) * P],
                                         qT[p][h2][64:128, bb * P:(bb + 1) * P],
                                         start=True, stop=True,
                                         tile_position=(64, 0))
                    pending.append((bb, expT, sce, sco))
                    expTs[bb] = expT
                    if len(pending) == 2 or bb == 3:
                        for _bb, _e, _sce, _sco in pending:
                            nc.scalar.activation(_e[:, 0:512], _sce, AF.Exp,
                                                 bias=zero_t, scale=1.0)
                            nc.scalar.activation(_e[:, 512:1024], _sco, AF.Exp,
                                                 bias=zero_t, scale=1.0)
                        pending = []
                if fillerA is not None:
                    fillerA()
                # Per bb: colsum matmuls, then DVE reciprocal_approx_fast
                # (~18 correct bits; keeps ACT on one table set), then ONE
                # fused [P,1024] normalize multiply. Tight per-bb emission so
                # the latency-critical softmax chain is never queued behind
                # bulk DVE work in the engine FIFO.
                for bb in range(4):
                    ddr2 = tpool.tile([P, TB], f32, tag="ddr", name="ddr",
                                      bufs=3)
                    for j in range(2):
                        sl = slice(j * 512, (j + 1) * 512)
                        dsum = ps_sc.tile([P, 512], f32, tag="sc", name="sc")
                        nc.tensor.matmul(dsum, ones128b, expTs[bb][:, sl],
                                         start=True, stop=True)
                        nc.vector.reciprocal_approx_fast(out=ddr2[:, sl],
                                                         in_=dsum)
                    nc.vector.tensor_tensor(expTs[bb], expTs[bb], ddr2,
                                            op=OP.mult)
                if fillerB is not None:
                    fillerB()
                x3 = xres[h2].rearrange("p (k n) -> p k n", k=NC)
                for bb in range(4):
                    b = h2 * 4 + bb
                    ao = ps_ao.tile([P, 512], f32, tag="ao", name="ao")
                    for p in range(NPAIR):
                        for j in range(2):
                            h = 2 * p + j
                            pos = (h % 2) * 512 + (h // 2) * P
                            nc.tensor.matmul(ao[j * 64:(j + 1) * 64, p * P:(p + 1) * P],
                                             vl[b][:, h * 64:(h + 1) * 64],
                                             expTs[bb][:, pos:pos + P],
                                             start=True, stop=True,
                                             tile_position=(0, j * 64))
                    dst = x3[:, :, bb * P:(bb + 1) * P]
                    nc.vector.scalar_tensor_tensor(dst,
                                                   ao.rearrange("p (k n) -> p k n", k=NC),
                                                   0.0, dst,
                                                   op0=OP.add, op1=OP.add,
                                                   accum_out=ST[:, bb, 0:1])

            def ln_stats(ST, h2, nsum=1):
                # DVE part of layernorm: per-item sums were accumulated into
                # ST[:, b, 0:nsum] by the residual-evict ops; a fused
                # square+reduce per item fills ST[:, b, 4].
                x3 = xres[h2].rearrange("p (k n) -> p k n", k=NC)
                for bb in range(4):
                    sq = tpool.tile([P, 512], bf16, tag="sq", name="sq")
                    src = x3[:, :, bb * P:(bb + 1) * P]
                    nc.vector.scalar_tensor_tensor(
                        sq.rearrange("p (k n) -> p k n", k=NC), src, 1.0, src,
                        op0=OP.mult, op1=OP.mult,
                        accum_out=ST[:, bb, 4:5])
                if nsum == 1:
                    # cols 1-3 unwritten in attention mode; zero them so the
                    # stats matmul never reads uninitialized sbuf
                    nc.vector.memset(ST[:, :, 1:4], 0.0)

            def ln_apply(ST, h2, last=False, nsum=1):
                # PE partition-reduce + normalize. 1/sqrt(var+eps) via the
                # quake bit-trick + 2 Newton steps, all on DVE — the ACT
                # engine only ever sees Exp/Identity/Relu/Square, so a single
                # activation table set serves the whole kernel (no reloads).
                x3 = xres[h2].rearrange("p (k n) -> p k n", k=NC)
                tot = ps_pj.tile([P, 20], f32, tag="pj", name="pj")
                nc.tensor.matmul(tot, onesN,
                                 ST.rearrange("p a b -> p (a b)"),
                                 start=True, stop=True)
                tot3 = tot.rearrange("p (a b) -> p a b", b=5)
                mm_ = tpool.tile([P, 4], f32, tag="mm_", name="mm_")
                if nsum == 1:
                    nc.vector.tensor_copy(out=mm_, in_=tot3[:, :, 0])
                else:
                    nc.vector.reduce_sum(mm_, tot3[:, :, 0:nsum],
                                         axis=mybir.AxisListType.X)
                var = tpool.tile([P, 4], f32, tag="var", name="var")
                nc.vector.tensor_tensor(var, mm_, mm_, op=OP.mult)
                # var = (sumsq + eps) - mean^2
                nc.vector.scalar_tensor_tensor(var, tot3[:, :, 4], EPS, var,
                                               op0=OP.add, op1=OP.subtract)
                rsi = tpool.tile([P, 4], mybir.dt.int32, tag="rsi", name="rsi")
                nc.vector.tensor_scalar(rsi, var.bitcast(mybir.dt.int32), 1,
                                        None, op0=OP.logical_shift_right)
                # K - x computed as ~x + (K+1); walrus refuses mixed
                # bitwise/arith op pairs in one instruction, so split
                nc.vector.tensor_scalar(rsi, rsi, -1, None, op0=OP.bitwise_xor)
                nc.vector.tensor_scalar(rsi, rsi, 0x5f3759e0, None, op0=OP.add)
                r0 = rsi.bitcast(f32)
                rt = tpool.tile([P, 4], f32, tag="rt", name="rt")
                ru = tpool.tile([P, 4], f32, tag="ru", name="ru")
                rr = tpool.tile([P, 4], f32, tag="rr", name="rr")
                nc.vector.tensor_tensor(rt, var, r0, op=OP.mult)
                nc.vector.tensor_tensor(rt, rt, r0, op=OP.mult)
                nc.vector.tensor_scalar(ru, rt, -0.5, 1.5, op0=OP.mult, op1=OP.add)
                nc.vector.tensor_tensor(rr, r0, ru, op=OP.mult)
                nc.vector.tensor_tensor(rt, var, rr, op=OP.mult)
                nc.vector.tensor_tensor(rt, rt, rr, op=OP.mult)
                nc.vector.tensor_scalar(ru, rt, -0.5, 1.5, op0=OP.mult, op1=OP.add)
                nc.vector.tensor_tensor(rr, rr, ru, op=OP.mult)
                # nm = -mean*rr so the fp32 xres update can run on ACT as
                # Identity(x*rr + nm), offloading DVE
                nm = tpool.tile([P, 4], f32, tag="nm", name="nm")
                nc.vector.tensor_tensor(nm, mm_, rr, op=OP.mult)
                nc.vector.tensor_scalar(nm, nm, -1.0, None, op0=OP.mult)
                # bf16 xn copy on DVE first (it gates the next phase's
                # matmuls); the fp32 xres in-place update runs on ACT
                xn3 = xn[h2].rearrange("p (k n) -> p k n", k=NC)
                for bb in range(4):
                    src = x3[:, :, bb * P:(bb + 1) * P]
                    if not last:
                        nc.vector.tensor_scalar(xn3[:, :, bb * P:(bb + 1) * P],
                                                src, mm_[:, bb:bb + 1],
                                                rr[:, bb:bb + 1],
                                                op0=OP.subtract, op1=OP.mult)
                for bb in range(4):
                    src = x3[:, :, bb * P:(bb + 1) * P]
                    nc.scalar.activation(src, src, AF.Identity,
                                         bias=nm[:, bb:bb + 1],
                                         scale=rr[:, bb:bb + 1])

            def out_dma(h):
                for k in range(NC):
                    nc.sync.dma_start(
                        out=d_out[k * P:(k + 1) * P, h * 512:(h + 1) * 512],
                        in_=xres[h][:, k * 512:(k + 1) * 512])

            for l in range(n_layers):
                wts_next = load_attn_w(l + 1) if l + 1 < n_layers else None
                if wts_next is not None:
                    bvs_n, bvc_n = load_bv(l + 1)

                def new_st():
                    return tpool.tile([P, 4, 5], f32, tag="ST", name="ST", bufs=8)

                qcol = lambda mc, ml=l: (ml * 4 + 0) * 4 + mc
                kcol = lambda mc, ml=l: (ml * 4 + 1) * 4 + mc
                cqcol = lambda mc, ml=l: (ml * 4 + 2) * 4 + mc
                ckcol = lambda mc, ml=l: (ml * 4 + 3) * 4 + mc

                # --- QKV projections (wq/wk half 0 hoisted into layer l-1) ---
                ST1 = [new_st(), new_st()]
                if l == 0:
                    proj_qk(qT, wts["wq"], xn, qcol, 0.125)
                    proj_qk(kT, wts["wk"], xn, kcol, 1.0)
                    proj_v(vS, wts["wv"], xn, bvs, range(BL))
                else:
                    proj_v(vS, wts["wv"], xn, bvs, range(0, 4))
                    proj_qk(qT, wts["wq"], xn, qcol, 0.125, halves=(1,))
                    proj_qk(kT, wts["wk"], xn, kcol, 1.0)
                    proj_v(vS, wts["wv"], xn, bvs, range(4, 8))

                # --- self attention (cross K/V emitted as PE filler) ---
                def fA_sa0():
                    proj_qk(kcT, wts["ck"], eTs, ckcol, 1.0, halves=(0,))

                def fB_sa0():
                    proj_v(vC, wts["cv"], eTs, bvc, range(0, 4))

                def fA_sa1():
                    proj_qk(kcT, wts["ck"], eTs, ckcol, 1.0, halves=(1,))

                def fB_sa1():
                    proj_v(vC, wts["cv"], eTs, bvc, range(4, 8))

                attention_half(kT, vS, ST1[0], 0, fillerA=fA_sa0, fillerB=fB_sa0)
                ln_stats(ST1[0], 0)
                ln_apply(ST1[0], 0)
                attention_half(kT, vS, ST1[1], 1, fillerA=fA_sa1, fillerB=fB_sa1)
                ln_stats(ST1[1], 1)
                proj_qk(qT, wts["cq"], xn, cqcol, 0.125, halves=(0,))
                ln_apply(ST1[1], 1)

                # --- cross attention ---
                ST2 = [new_st(), new_st()]

                def fA_ca0():
                    proj_qk(qT, wts["cq"], xn, cqcol, 0.125, halves=(1,))

                def ffn_w1(h2, fcs):
                    for fc in fcs:
                        pj = ps_pj.tile([P, 512], f32, tag="pj", name="pj")
                        for kc in range(NC):
                            nc.tensor.matmul(pj, w1s[:, kc, fc * P:(fc + 1) * P],
                                             xn[h2][:, xsl(h2, kc)],
                                             start=(kc == 0), stop=(kc == NC - 1))
                        col = l * NF + fc
                        nc.scalar.activation(hT[fc], pj, AF.Relu,
                                             bias=b1_s[:, col:col + 1], scale=1.0)

                def fA_ca1():
                    ffn_w1(0, range(0, 3))

                def fB_ca1():
                    ffn_w1(0, range(3, 8))

                attention_half(kcT, vC, ST2[0], 0, fillerA=fA_ca0)
                ln_stats(ST2[0], 0)
                ln_apply(ST2[0], 0)
                attention_half(kcT, vC, ST2[1], 1, fillerA=fA_ca1, fillerB=fB_ca1)
                ln_stats(ST2[1], 1)
                ffn_w1(0, range(8, NF))
                ln_apply(ST2[1], 1)

                # --- feed-forward ---
                ST3 = [new_st(), new_st()]

                def ffn_w2(h2, ST):
                    for mc in range(NC):
                        pj = ps_pj.tile([P, 512], f32, tag="pj", name="pj")
                        for fc in range(NF):
                            nc.tensor.matmul(pj, w2s[:, fc, mc * P:(mc + 1) * P],
                                             hT[fc],
                                             start=(fc == 0), stop=(fc == NF - 1))
                        b2col = b2_s[:, l * NC + mc:l * NC + mc + 1]
                        for bb in range(4):
                            dst = xres[h2][:, xsl(h2, mc, bb)]
                            nc.vector.scalar_tensor_tensor(dst,
                                                           pj[:, bb * P:(bb + 1) * P],
                                                           b2col, dst,
                                                           op0=OP.add, op1=OP.add,
                                                           accum_out=ST[:, bb, mc:mc + 1])

                last = l == n_layers - 1
                ffn_w2(0, ST3[0])
                ln_stats(ST3[0], 0)
                ffn_w1(1, range(0, 4))
                ln_apply(ST3[0], 0, last=last)
                if last:
                    out_dma(0)
                ffn_w1(1, range(4, NF))
                if l + 1 < n_layers:
                    w1s_n, w2s_n = load_ffn_w(l + 1)
                ffn_w2(1, ST3[1])
                ln_stats(ST3[1], 1)
                if wts_next is not None:
                    # hoist: next layer's Q projection (half 0) keeps the PE
                    # busy while the final layernorm's DVE/ACT chain runs
                    proj_qk(qT, wts_next["wq"], xn,
                            lambda mc, ml=l + 1: (ml * 4 + 0) * 4 + mc,
                            0.125, halves=(0,))
                ln_apply(ST3[1], 1, last=last)
                if last:
                    out_dma(1)
                else:
                    wts = wts_next
                    bvs, bvc = bvs_n, bvc_n
                    w1s, w2s = w1s_n, w2s_n

    nc.compile()
    return nc


def _prep_shared(inputs):
    """Host-side weight repacking (shared across cores)."""
    def packw(w):  # [L,H,C,DH] -> [L,128,NC,H*DH]  (c = kc*128+p)
        w2 = np.ascontiguousarray(w.transpose(0, 2, 1, 3)).reshape(L, C, HD)
        return np.ascontiguousarray(
            w2.reshape(L, NC, P, HD).transpose(0, 2, 1, 3)).astype(_BF)

    shared = {}
    for nm, key in (("wq", "sa_wq"), ("wk", "sa_wk"), ("wv", "sa_wv"),
                    ("cq", "ca_wq"), ("ck", "ca_wk"), ("cv", "ca_wv")):
        shared[nm] = packw(inputs[key])
    shared["w1"] = np.ascontiguousarray(
        inputs["ff_w1"].reshape(L, NC, P, FF).transpose(0, 2, 1, 3)).astype(_BF)
    shared["w2"] = np.ascontiguousarray(
        inputs["ff_w2"].reshape(L, NF, P, C).transpose(0, 2, 1, 3)).astype(_BF)

    bqk = np.zeros((P, L * 16), np.float32)
    for l in range(L):
        for mi, (bias, s) in enumerate((
                (inputs["sa_bq"][l], 0.125), (inputs["sa_bk"][l], 1.0),
                (inputs["ca_bq"][l], 0.125), (inputs["ca_bk"][l], 1.0))):
            flat = bias.reshape(HD).astype(np.float32) * s
            for mc in range(NC):
                bqk[:, (l * 4 + mi) * 4 + mc] = flat[mc * P:(mc + 1) * P]
    shared["bqk"] = bqk

    bv = np.stack([inputs["sa_bv"].reshape(L, HD),
                   inputs["ca_bv"].reshape(L, HD)], axis=1).astype(np.float32)
    shared["bvb"] = np.ascontiguousarray(
        np.broadcast_to(bv[:, :, None, :], (L, 2, P, HD)))

    b1 = np.zeros((P, L * NF), np.float32)
    for l in range(L):
        for fc in range(NF):
            b1[:, l * NF + fc] = inputs["ff_b1"][l, fc * P:(fc + 1) * P]
    shared["b1"] = b1
    b2 = np.zeros((P, L * NC), np.float32)
    for l in range(L):
        for mc in range(NC):
            b2[:, l * NC + mc] = inputs["ff_b2"][l, mc * P:(mc + 1) * P]
    shared["b2"] = b2
    return shared


LAST_RESULT = None


def _install_ntff_hook():
    """Register the axon NTFF profile hook that the image's antenv lacks.

    Only used for local benchmarking (KERNEL_TRACE=1); inert otherwise.
    """
    import sys
    import types
    try:
        import antenv
        if getattr(antenv, "axon_hooks", None) is not None:
            return
        from trn_agent_boot.trn_boot import _ntff_profile_via_ctypes
        mod = types.ModuleType("antenv.axon_hooks")
        mod._hook = _ntff_profile_via_ctypes("/opt/axon/libaxon_pjrt.so")

        def get_axon_ntff_profile_hook():
            return mod._hook

        def set_axon_ntff_profile_hook(h):
            mod._hook = h

        mod.get_axon_ntff_profile_hook = get_axon_ntff_profile_hook
        mod.set_axon_ntff_profile_hook = set_axon_ntff_profile_hook
        sys.modules["antenv.axon_hooks"] = mod
        antenv.axon_hooks = mod
    except Exception as e:  # pragma: no cover - profiling is best-effort
        print(f"ntff hook install failed: {e}")


def kernel(**inputs):
    global LAST_RESULT
    import os
    inputs = {k: np.asarray(v) for k, v in inputs.items()}
    if "nc" not in _cache:
        _cache["nc"] = _build()
    nc = _cache["nc"]

    shared = _prep_shared(inputs)
    x = inputs["x"].astype(np.float32)
    enc = inputs["encoder_output"].astype(np.float32)

    in_maps = []
    for core in range(NCORES):
        sl = slice(core * BL, (core + 1) * BL)
        xT = np.ascontiguousarray(x[sl].transpose(2, 0, 1)).reshape(C, TB)
        eT = np.ascontiguousarray(enc[sl].transpose(2, 0, 1)).reshape(C, TB)
        m = dict(shared)
        m["xT"] = xT
        m["xTb"] = xT.astype(_BF)
        m["eT"] = eT.astype(_BF)
        in_maps.append(m)

    trace = bool(int(os.environ.get("KERNEL_TRACE", "0")))
    if trace:
        _install_ntff_hook()
    from concourse.bass_utils import run_bass_kernel_spmd
    res = run_bass_kernel_spmd(nc, in_maps, list(range(NCORES)), trace=trace,
                               trace_cores=[0])
    LAST_RESULT = res

    out = np.empty((B, T, C), np.float32)
    for core in range(NCORES):
        outT = res.results[core]["outT"]  # [C, TB]
        out[core * BL:(core + 1) * BL] = outT.reshape(C, BL, T).transpose(1, 2, 0)
    return out


# revision 23
# speedup vs baseline: 1.1792x; 1.1227x over previous
"""Trainium2 Bass kernel for a 6-layer transformer decoder (self+cross attention).

Strategy: data-parallel over batch across 8 NeuronCores. Each core runs the
full decoder on its 8-batch-item shard, with activations kept transposed
[C, B_local*T] in SBUF so every projection is a natural lhsT.T @ rhs matmul
with a 512-wide moving dim. Matmul inputs are bf16 (fp32 PSUM accumulate);
residual stream and layernorm statistics stay fp32.

Scheduling (v2): attention weights double-buffered and prefetched a layer
ahead; filler projections emitted BEFORE the softmax colsum so the PE queue
never head-of-line blocks on the ACT exp chain; the next layer's Q
projection (half 0) is hoisted across the layer boundary to cover the final
layernorm's DVE/ACT critical path; startup DMAs ordered by first use.
"""

import numpy as np
import ml_dtypes

L, H, C, DH, FF = 6, 8, 512, 64, 2048
B, T = 64, 128
EPS = 1e-5
NCORES = 8
BL = B // NCORES          # batch items per core
TB = BL * T               # 1024 activation columns per core
NC = C // 128             # 4 channel chunks
NF = FF // 128            # 16 ff chunks
NPAIR = H // 2            # head pairs
HD = H * DH               # 512
P = 128
NORM = 1.0 / (T * C)      # layernorm 1/N, folded into the stats matmul

_BF = ml_dtypes.bfloat16

_cache = {}


def _build(n_layers=L):
    from contextlib import ExitStack

    import concourse.bass as bass  # noqa: F401
    import concourse.tile as tile
    import concourse.mybir as mybir
    from concourse import bacc

    dt = mybir.dt
    AF = mybir.ActivationFunctionType
    OP = mybir.AluOpType
    f32, bf16 = dt.float32, dt.bfloat16

    nc = bacc.Bacc("TRN2", target_bir_lowering=False, debug=False, num_devices=NCORES)

    def act_recip(out, in_):
        # ACT-engine reciprocal (~1e-5 rel err measured on hw for this value
        # range); bass's wrapper refuses Reciprocal so emit directly. Runs on
        # ACT so the DVE stays free for the softmax normalize multiplies.
        nc.scalar.add_instruction(mybir.InstActivation(
            name=nc.get_next_instruction_name(),
            func=AF.Reciprocal,
            ins=[nc.scalar.lower_ap(in_),
                 mybir.ImmediateValue(dtype=f32, value=0.0),
                 mybir.ImmediateValue(dtype=f32, value=1.0),
                 mybir.ImmediateValue(dtype=f32, value=0.0)],
            outs=[nc.scalar.lower_ap(out)],
        ))

    d_xT = nc.dram_tensor("xT", [C, TB], f32, kind="ExternalInput").ap()
    d_xTb = nc.dram_tensor("xTb", [C, TB], bf16, kind="ExternalInput").ap()
    d_eT = nc.dram_tensor("eT", [C, TB], bf16, kind="ExternalInput").ap()
    d_w = {}
    for name in ("wq", "wk", "wv", "cq", "ck", "cv"):
        d_w[name] = nc.dram_tensor(name, [L, P, NC, HD], bf16, kind="ExternalInput").ap()
    d_w1 = nc.dram_tensor("w1", [L, P, NC, FF], bf16, kind="ExternalInput").ap()
    d_w2 = nc.dram_tensor("w2", [L, P, NF, C], bf16, kind="ExternalInput").ap()
    d_bqk = nc.dram_tensor("bqk", [P, L * 16], f32, kind="ExternalInput").ap()
    d_bvb = nc.dram_tensor("bvb", [L, 2, P, HD], f32, kind="ExternalInput").ap()
    d_b1 = nc.dram_tensor("b1", [P, L * NF], f32, kind="ExternalInput").ap()
    d_b2 = nc.dram_tensor("b2", [P, L * NC], f32, kind="ExternalInput").ap()
    d_out = nc.dram_tensor("outT", [C, TB], f32, kind="ExternalOutput").ap()

    with tile.TileContext(nc) as tc:
        with ExitStack() as ctx:
            cpool = ctx.enter_context(tc.tile_pool(name="const", bufs=1))
            apool = ctx.enter_context(tc.tile_pool(name="acts", bufs=1))
            wpool = ctx.enter_context(tc.tile_pool(name="wts", bufs=1))
            tpool = ctx.enter_context(tc.tile_pool(name="tmp", bufs=2))
            ps_pj = ctx.enter_context(tc.tile_pool(name="pj", bufs=3, space="PSUM"))
            ps_sc = ctx.enter_context(tc.tile_pool(name="sc", bufs=3, space="PSUM"))
            ps_ao = ctx.enter_context(tc.tile_pool(name="ao", bufs=2, space="PSUM"))

            # ---- constants (engine memsets; no DMA) ----
            ones128b = cpool.tile([P, P], bf16, tag="ones128b")
            nc.vector.memset(ones128b, 1.0)
            onesN = cpool.tile([P, P], f32, tag="onesN")
            nc.vector.memset(onesN, NORM)      # ones/65536 for LN stats matmul
            eps_t = cpool.tile([P, 1], f32, tag="eps")
            nc.vector.memset(eps_t, EPS)
            zero_t = cpool.tile([P, 1], f32, tag="zero")
            nc.vector.memset(zero_t, 0.0)

            # ---- persistent activations (kc-major merged tiles per half) ----
            xres = [apool.tile([P, NC * 512], f32, tag=f"xres{h}", name=f"xres{h}")
                    for h in range(2)]
            xn = [apool.tile([P, NC * 512], bf16, tag=f"xn{h}", name=f"xn{h}")
                  for h in range(2)]
            eTs = [apool.tile([P, NC * 512], bf16, tag=f"eT{h}", name=f"eT{h}")
                   for h in range(2)]

            def dma_act(dst_list, src, h, dtype_slice=True):
                for k in range(NC):
                    rs = slice(k * P, (k + 1) * P)
                    cs_ = slice(h * 512, (h + 1) * 512)
                    ts_ = slice(k * 512, (k + 1) * 512)
                    nc.sync.dma_start(out=dst_list[h][:, ts_], in_=src[rs, cs_])

            # startup DMAs ordered by first use: xn (first projections) first,
            # then SA weights, encoder half 0 + ck (SA-h0 fillers), residual
            # half 0 (SA-h0 attn-out evict), then the rest.
            dma_act(xn, d_xTb, 0)
            dma_act(xn, d_xTb, 1)
            bqk_s = cpool.tile([P, L * 16], f32, tag="bqk")
            nc.sync.dma_start(out=bqk_s, in_=d_bqk)

            # warm-up during the input-DMA window: ~4us of dummy matmuls on
            # the const tile flips the PE HAM clock-gate to 8/8 before the
            # first real projection, and a dummy exp pulls the one ACT
            # table load off the critical path.
            wu_t = tpool.tile([P, 1], f32, tag="wu", name="wu")
            nc.scalar.activation(wu_t, eps_t, AF.Exp, bias=zero_t, scale=1.0)
            for wi in range(40):
                wup = ps_pj.tile([P, P], f32, tag="pj", name="pj")
                nc.tensor.matmul(wup, ones128b, ones128b, start=True, stop=True)

            W_ATT = ("wq", "wk", "wv", "cq", "ck", "cv")

            def load_attn_w(l, names=W_ATT):
                wts = {}
                for name in names:
                    w = wpool.tile([P, NC, HD], bf16, tag=name, name=name, bufs=2)
                    nc.sync.dma_start(out=w, in_=d_w[name][l])
                    wts[name] = w
                return wts

            def load_bv(l):
                bvs = wpool.tile([P, HD], f32, tag="bvs", name="bvs", bufs=2)
                nc.sync.dma_start(out=bvs, in_=d_bvb[l, 0])
                bvc = wpool.tile([P, HD], f32, tag="bvc", name="bvc", bufs=2)
                nc.sync.dma_start(out=bvc, in_=d_bvb[l, 1])
                return bvs, bvc

            def load_ffn_w(l):
                w1s = wpool.tile([P, NC, FF], bf16, tag="w1", name="w1")
                nc.sync.dma_start(out=w1s, in_=d_w1[l])
                w2s = wpool.tile([P, NF, C], bf16, tag="w2", name="w2")
                nc.sync.dma_start(out=w2s, in_=d_w2[l])
                return w1s, w2s

            wts = load_attn_w(0, names=("wq", "wk", "wv"))
            dma_act(eTs, d_eT, 0)
            wts.update(load_attn_w(0, names=("ck",)))
            dma_act(xres, d_xT, 0)
            wts.update(load_attn_w(0, names=("cv",)))
            bvs, bvc = load_bv(0)
            wts.update(load_attn_w(0, names=("cq",)))
            dma_act(eTs, d_eT, 1)
            dma_act(xres, d_xT, 1)
            w1s, w2s = load_ffn_w(0)
            b1_s = cpool.tile([P, L * NF], f32, tag="b1")
            nc.sync.dma_start(out=b1_s, in_=d_b1)
            b2_s = cpool.tile([P, L * NC], f32, tag="b2")
            nc.sync.dma_start(out=b2_s, in_=d_b2)

            def xsl(h2, kc, bb=None):
                if bb is None:
                    return slice(kc * 512, (kc + 1) * 512)
                return slice(kc * 512 + bb * P, kc * 512 + (bb + 1) * P)

            qT = [[apool.tile([P, 512], bf16, tag=f"qT{k}_{h}", name=f"qT{k}_{h}")
                   for h in range(2)] for k in range(NC)]
            kT = [[apool.tile([P, 512], bf16, tag=f"kT{k}_{h}", name=f"kT{k}_{h}")
                   for h in range(2)] for k in range(NC)]
            kcT = [[apool.tile([P, 512], bf16, tag=f"kcT{k}_{h}", name=f"kcT{k}_{h}")
                    for h in range(2)] for k in range(NC)]
            vS = [apool.tile([P, HD], bf16, tag=f"v{b}", name=f"v{b}") for b in range(BL)]
            vC = [apool.tile([P, HD], bf16, tag=f"vc{b}", name=f"vc{b}") for b in range(BL)]
            hT = [apool.tile([P, 512], bf16, tag=f"hT{f}", name=f"hT{f}") for f in range(NF)]

            def proj_qk(dst, wt, src, col_of, scale, halves=(0, 1), evict="act"):
                for h2 in halves:
                    for mc in range(NC):
                        pj = ps_pj.tile([P, 512], f32, tag="pj", name="pj")
                        for kc in range(NC):
                            nc.tensor.matmul(pj, wt[:, kc, mc * P:(mc + 1) * P],
                                             src[h2][:, xsl(h2, kc)],
                                             start=(kc == 0), stop=(kc == NC - 1))
                        col = col_of(mc)
                        bias = bqk_s[:, col:col + 1]
                        if evict == "act":
                            nc.scalar.activation(dst[mc][h2], pj, AF.Identity,
                                                 bias=bias, scale=scale)
                        elif scale == 1.0:
                            # DVE evict keeps the ACT queue free for the
                            # softmax exp/recip chain inside attention
                            nc.vector.tensor_scalar(dst[mc][h2], pj, bias, None,
                                                    op0=OP.add)
                        else:
                            nc.vector.tensor_scalar(dst[mc][h2], pj, scale, bias,
                                                    op0=OP.mult, op1=OP.add)

            def proj_v(dst, wt, src, bias_tile, bs):
                for b in bs:
                    h2, bb = divmod(b, 4)
                    pj = ps_pj.tile([P, 512], f32, tag="pj", name="pj")
                    for kc in range(NC):
                        nc.tensor.matmul(pj, src[h2][:, xsl(h2, kc, bb)],
                                         wt[:, kc, :],
                                         start=(kc == 0), stop=(kc == NC - 1))
                    nc.vector.tensor_tensor(dst[b], pj, bias_tile, op=OP.add)

            def attention_half(kTl, vl, ST, h2, fillerA=None, fillerB=None):
                # Per half (4 batch items). ACT ops batched by function; PE
                # filler work emitted BEFORE the colsum matmuls so the PE
                # queue doesn't head-of-line block on the ACT exp chain.
                expTs = {}
                pending = []
                for bb in range(4):
                    expT = tpool.tile([P, TB], bf16, tag="expT", name="expT",
                                      bufs=4)
                    sce = ps_sc.tile([P, 512], f32, tag="sc", name="sc")
                    sco = ps_sc.tile([P, 512], f32, tag="sc", name="sc")
                    for p in range(NPAIR):
                        nc.tensor.matmul(sce[:, p * P:(p + 1) * P],
                                         kTl[p][h2][0:64, bb * P:(bb + 1) * P],
                                         qT[p][h2][0:64, bb * P:(bb + 1) * P],
                                         start=True, stop=True,
                                         tile_position=(0, 0))
                        nc.tensor.matmul(sco[:, p * P:(p + 1) * P],
                                         kTl[p][h2][64:128, bb * P:(bb + 1) * P],
                                         qT[p][h2][64:128, bb * P:(bb + 1) * P],
                                         start=True, stop=True,
                                         tile_position=(64, 0))
                    pending.append((bb, expT, sce, sco))
                    expTs[bb] = expT
                    if len(pending) == 2 or bb == 3:
                        for _bb, _e, _sce, _sco in pending:
                            nc.scalar.activation(_e[:, 0:512], _sce, AF.Exp,
                                                 bias=zero_t, scale=1.0)
                            nc.scalar.activation(_e[:, 512:1024], _sco, AF.Exp,
                                                 bias=zero_t, scale=1.0)
                        pending = []
                if fillerA is not None:
                    fillerA()
                # colsum broadcast matmuls, then all ACT reciprocals (batched
                # so the exp->recip table swap happens once), then DVE mults
                ddrs = {}
                for bb in range(4):
                    for j in range(2):
                        sl = slice(j * 512, (j + 1) * 512)
                        dsum = ps_sc.tile([P, 512], f32, tag="sc", name="sc")
                        nc.tensor.matmul(dsum, ones128b, expTs[bb][:, sl],
                                         start=True, stop=True)
                        ddr = tpool.tile([P, 512], bf16, tag="ddr", name="ddr",
                                         bufs=6)
                        act_recip(ddr, dsum)
                        ddrs[(bb, j)] = ddr
                if fillerB is not None:
                    fillerB()
                for bb in range(4):
                    for j in range(2):
                        sl = slice(j * 512, (j + 1) * 512)
                        nc.vector.tensor_tensor(expTs[bb][:, sl],
                                                expTs[bb][:, sl],
                                                ddrs[(bb, j)], op=OP.mult)
                x3 = xres[h2].rearrange("p (k n) -> p k n", k=NC)
                for bb in range(4):
                    b = h2 * 4 + bb
                    ao = ps_ao.tile([P, 512], f32, tag="ao", name="ao")
                    for p in range(NPAIR):
                        for j in range(2):
                            h = 2 * p + j
                            pos = (h % 2) * 512 + (h // 2) * P
                            nc.tensor.matmul(ao[j * 64:(j + 1) * 64, p * P:(p + 1) * P],
                                             vl[b][:, h * 64:(h + 1) * 64],
                                             expTs[bb][:, pos:pos + P],
                                             start=True, stop=True,
                                             tile_position=(0, j * 64))
                    dst = x3[:, :, bb * P:(bb + 1) * P]
                    nc.vector.scalar_tensor_tensor(dst,
                                                   ao.rearrange("p (k n) -> p k n", k=NC),
                                                   0.0, dst,
                                                   op0=OP.add, op1=OP.add,
                                                   accum_out=ST[:, bb, 0:1])

            def ln_stats(ST, h2, nsum=1):
                # DVE part of layernorm: per-item sums were accumulated into
                # ST[:, b, 0:nsum] by the residual-evict ops; a fused
                # square+reduce per item fills ST[:, b, 4].
                x3 = xres[h2].rearrange("p (k n) -> p k n", k=NC)
                for bb in range(4):
                    sq = tpool.tile([P, 512], bf16, tag="sq", name="sq")
                    src = x3[:, :, bb * P:(bb + 1) * P]
                    nc.vector.scalar_tensor_tensor(
                        sq.rearrange("p (k n) -> p k n", k=NC), src, 1.0, src,
                        op0=OP.mult, op1=OP.mult,
                        accum_out=ST[:, bb, 4:5])
                if nsum == 1:
                    # cols 1-3 unwritten in attention mode; zero them so the
                    # stats matmul never reads uninitialized sbuf
                    nc.vector.memset(ST[:, :, 1:4], 0.0)

            def ln_apply(ST, h2, last=False, nsum=1):
                # PE partition-reduce + normalize. 1/sqrt(var+eps) via the
                # quake bit-trick + 2 Newton steps, all on DVE — the ACT
                # engine only ever sees Exp/Identity/Relu/Square, so a single
                # activation table set serves the whole kernel (no reloads).
                x3 = xres[h2].rearrange("p (k n) -> p k n", k=NC)
                tot = ps_pj.tile([P, 20], f32, tag="pj", name="pj")
                nc.tensor.matmul(tot, onesN,
                                 ST.rearrange("p a b -> p (a b)"),
                                 start=True, stop=True)
                tot3 = tot.rearrange("p (a b) -> p a b", b=5)
                mm_ = tpool.tile([P, 4], f32, tag="mm_", name="mm_")
                if nsum == 1:
                    nc.vector.tensor_copy(out=mm_, in_=tot3[:, :, 0])
                else:
                    nc.vector.reduce_sum(mm_, tot3[:, :, 0:nsum],
                                         axis=mybir.AxisListType.X)
                var = tpool.tile([P, 4], f32, tag="var", name="var")
                nc.vector.tensor_tensor(var, mm_, mm_, op=OP.mult)
                # var = (sumsq + eps) - mean^2
                nc.vector.scalar_tensor_tensor(var, tot3[:, :, 4], EPS, var,
                                               op0=OP.add, op1=OP.subtract)
                rsi = tpool.tile([P, 4], mybir.dt.int32, tag="rsi", name="rsi")
                nc.vector.tensor_scalar(rsi, var.bitcast(mybir.dt.int32), 1,
                                        None, op0=OP.logical_shift_right)
                # K - x computed as ~x + (K+1); walrus refuses mixed
                # bitwise/arith op pairs in one instruction, so split
                nc.vector.tensor_scalar(rsi, rsi, -1, None, op0=OP.bitwise_xor)
                nc.vector.tensor_scalar(rsi, rsi, 0x5f3759e0, None, op0=OP.add)
                r0 = rsi.bitcast(f32)
                rt = tpool.tile([P, 4], f32, tag="rt", name="rt")
                ru = tpool.tile([P, 4], f32, tag="ru", name="ru")
                rr = tpool.tile([P, 4], f32, tag="rr", name="rr")
                nc.vector.tensor_tensor(rt, var, r0, op=OP.mult)
                nc.vector.tensor_tensor(rt, rt, r0, op=OP.mult)
                nc.vector.tensor_scalar(ru, rt, -0.5, 1.5, op0=OP.mult, op1=OP.add)
                nc.vector.tensor_tensor(rr, r0, ru, op=OP.mult)
                nc.vector.tensor_tensor(rt, var, rr, op=OP.mult)
                nc.vector.tensor_tensor(rt, rt, rr, op=OP.mult)
                nc.vector.tensor_scalar(ru, rt, -0.5, 1.5, op0=OP.mult, op1=OP.add)
                nc.vector.tensor_tensor(rr, rr, ru, op=OP.mult)
                xn3 = xn[h2].rearrange("p (k n) -> p k n", k=NC)
                for bb in range(4):
                    src = x3[:, :, bb * P:(bb + 1) * P]
                    if not last:
                        # bf16 normalized copy first — unblocks the next
                        # phase's matmuls before the fp32 in-place update
                        nc.vector.tensor_scalar(xn3[:, :, bb * P:(bb + 1) * P],
                                                src, mm_[:, bb:bb + 1],
                                                rr[:, bb:bb + 1],
                                                op0=OP.subtract, op1=OP.mult)
                for bb in range(4):
                    src = x3[:, :, bb * P:(bb + 1) * P]
                    nc.vector.tensor_scalar(src, src, mm_[:, bb:bb + 1],
                                            rr[:, bb:bb + 1],
                                            op0=OP.subtract, op1=OP.mult)

            def out_dma(h):
                for k in range(NC):
                    nc.sync.dma_start(
                        out=d_out[k * P:(k + 1) * P, h * 512:(h + 1) * 512],
                        in_=xres[h][:, k * 512:(k + 1) * 512])

            for l in range(n_layers):
                wts_next = load_attn_w(l + 1) if l + 1 < n_layers else None
                if wts_next is not None:
                    bvs_n, bvc_n = load_bv(l + 1)

                def new_st():
                    return tpool.tile([P, 4, 5], f32, tag="ST", name="ST", bufs=8)

                qcol = lambda mc, ml=l: (ml * 4 + 0) * 4 + mc
                kcol = lambda mc, ml=l: (ml * 4 + 1) * 4 + mc
                cqcol = lambda mc, ml=l: (ml * 4 + 2) * 4 + mc
                ckcol = lambda mc, ml=l: (ml * 4 + 3) * 4 + mc

                # --- QKV projections (wq/wk half 0 hoisted into layer l-1) ---
                ST1 = [new_st(), new_st()]
                if l == 0:
                    proj_qk(qT, wts["wq"], xn, qcol, 0.125)
                    proj_qk(kT, wts["wk"], xn, kcol, 1.0)
                    proj_v(vS, wts["wv"], xn, bvs, range(BL))
                else:
                    proj_v(vS, wts["wv"], xn, bvs, range(0, 4))
                    proj_qk(qT, wts["wq"], xn, qcol, 0.125, halves=(1,))
                    proj_qk(kT, wts["wk"], xn, kcol, 1.0)
                    proj_v(vS, wts["wv"], xn, bvs, range(4, 8))

                # --- self attention (cross K/V emitted as PE filler) ---
                def fA_sa0():
                    proj_qk(kcT, wts["ck"], eTs, ckcol, 1.0, halves=(0,))

                def fB_sa0():
                    proj_v(vC, wts["cv"], eTs, bvc, range(0, 4))

                def fA_sa1():
                    proj_qk(kcT, wts["ck"], eTs, ckcol, 1.0, halves=(1,))

                def fB_sa1():
                    proj_v(vC, wts["cv"], eTs, bvc, range(4, 8))

                attention_half(kT, vS, ST1[0], 0, fillerA=fA_sa0, fillerB=fB_sa0)
                ln_stats(ST1[0], 0)
                ln_apply(ST1[0], 0)
                attention_half(kT, vS, ST1[1], 1, fillerA=fA_sa1, fillerB=fB_sa1)
                ln_stats(ST1[1], 1)
                proj_qk(qT, wts["cq"], xn, cqcol, 0.125, halves=(0,))
                ln_apply(ST1[1], 1)

                # --- cross attention ---
                ST2 = [new_st(), new_st()]

                def fA_ca0():
                    proj_qk(qT, wts["cq"], xn, cqcol, 0.125, halves=(1,))

                def ffn_w1(h2, fcs):
                    for fc in fcs:
                        pj = ps_pj.tile([P, 512], f32, tag="pj", name="pj")
                        for kc in range(NC):
                            nc.tensor.matmul(pj, w1s[:, kc, fc * P:(fc + 1) * P],
                                             xn[h2][:, xsl(h2, kc)],
                                             start=(kc == 0), stop=(kc == NC - 1))
                        col = l * NF + fc
                        nc.scalar.activation(hT[fc], pj, AF.Relu,
                                             bias=b1_s[:, col:col + 1], scale=1.0)

                def fA_ca1():
                    ffn_w1(0, range(0, 3))

                def fB_ca1():
                    ffn_w1(0, range(3, 8))

                attention_half(kcT, vC, ST2[0], 0, fillerA=fA_ca0)
                ln_stats(ST2[0], 0)
                ln_apply(ST2[0], 0)
                attention_half(kcT, vC, ST2[1], 1, fillerA=fA_ca1, fillerB=fB_ca1)
                ln_stats(ST2[1], 1)
                ffn_w1(0, range(8, NF))
                ln_apply(ST2[1], 1)

                # --- feed-forward ---
                ST3 = [new_st(), new_st()]

                def ffn_w2(h2, ST):
                    for mc in range(NC):
                        pj = ps_pj.tile([P, 512], f32, tag="pj", name="pj")
                        for fc in range(NF):
                            nc.tensor.matmul(pj, w2s[:, fc, mc * P:(mc + 1) * P],
                                             hT[fc],
                                             start=(fc == 0), stop=(fc == NF - 1))
                        b2col = b2_s[:, l * NC + mc:l * NC + mc + 1]
                        for bb in range(4):
                            dst = xres[h2][:, xsl(h2, mc, bb)]
                            nc.vector.scalar_tensor_tensor(dst,
                                                           pj[:, bb * P:(bb + 1) * P],
                                                           b2col, dst,
                                                           op0=OP.add, op1=OP.add,
                                                           accum_out=ST[:, bb, mc:mc + 1])

                last = l == n_layers - 1
                ffn_w2(0, ST3[0])
                ln_stats(ST3[0], 0)
                ffn_w1(1, range(0, 4))
                ln_apply(ST3[0], 0, last=last)
                if last:
                    out_dma(0)
                ffn_w1(1, range(4, NF))
                if l + 1 < n_layers:
                    w1s_n, w2s_n = load_ffn_w(l + 1)
                ffn_w2(1, ST3[1])
                ln_stats(ST3[1], 1)
                if wts_next is not None:
                    # hoist: next layer's Q projection (half 0) keeps the PE
                    # busy while the final layernorm's DVE/ACT chain runs
                    proj_qk(qT, wts_next["wq"], xn,
                            lambda mc, ml=l + 1: (ml * 4 + 0) * 4 + mc,
                            0.125, halves=(0,))
                ln_apply(ST3[1], 1, last=last)
                if last:
                    out_dma(1)
                else:
                    wts = wts_next
                    bvs, bvc = bvs_n, bvc_n
                    w1s, w2s = w1s_n, w2s_n

    nc.compile()
    return nc


def _prep_shared(inputs):
    """Host-side weight repacking (shared across cores)."""
    def packw(w):  # [L,H,C,DH] -> [L,128,NC,H*DH]  (c = kc*128+p)
        w2 = np.ascontiguousarray(w.transpose(0, 2, 1, 3)).reshape(L, C, HD)
        return np.ascontiguousarray(
            w2.reshape(L, NC, P, HD).transpose(0, 2, 1, 3)).astype(_BF)

    shared = {}
    for nm, key in (("wq", "sa_wq"), ("wk", "sa_wk"), ("wv", "sa_wv"),
                    ("cq", "ca_wq"), ("ck", "ca_wk"), ("cv", "ca_wv")):
        shared[nm] = packw(inputs[key])
    shared["w1"] = np.ascontiguousarray(
        inputs["ff_w1"].reshape(L, NC, P, FF).transpose(0, 2, 1, 3)).astype(_BF)
    shared["w2"] = np.ascontiguousarray(
        inputs["ff_w2"].reshape(L, NF, P, C).transpose(0, 2, 1, 3)).astype(_BF)

    bqk = np.zeros((P, L * 16), np.float32)
    for l in range(L):
        for mi, (bias, s) in enumerate((
                (inputs["sa_bq"][l], 0.125), (inputs["sa_bk"][l], 1.0),
                (inputs["ca_bq"][l], 0.125), (inputs["ca_bk"][l], 1.0))):
            flat = bias.reshape(HD).astype(np.float32) * s
            for mc in range(NC):
                bqk[:, (l * 4 + mi) * 4 + mc] = flat[mc * P:(mc + 1) * P]
    shared["bqk"] = bqk

    bv = np.stack([inputs["sa_bv"].reshape(L, HD),
                   inputs["ca_bv"].reshape(L, HD)], axis=1).astype(np.float32)
    shared["bvb"] = np.ascontiguousarray(
        np.broadcast_to(bv[:, :, None, :], (L, 2, P, HD)))

    b1 = np.zeros((P, L * NF), np.float32)
    for l in range(L):
        for fc in range(NF):
            b1[:, l * NF + fc] = inputs["ff_b1"][l, fc * P:(fc + 1) * P]
    shared["b1"] = b1
    b2 = np.zeros((P, L * NC), np.float32)
    for l in range(L):
        for mc in range(NC):
            b2[:, l * NC + mc] = inputs["ff_b2"][l, mc * P:(mc + 1) * P]
    shared["b2"] = b2
    return shared


LAST_RESULT = None


def _install_ntff_hook():
    """Register the axon NTFF profile hook that the image's antenv lacks.

    Only used for local benchmarking (KERNEL_TRACE=1); inert otherwise.
    """
    import sys
    import types
    try:
        import antenv
        if getattr(antenv, "axon_hooks", None) is not None:
            return
        from trn_agent_boot.trn_boot import _ntff_profile_via_ctypes
        mod = types.ModuleType("antenv.axon_hooks")
        mod._hook = _ntff_profile_via_ctypes("/opt/axon/libaxon_pjrt.so")

        def get_axon_ntff_profile_hook():
            return mod._hook

        def set_axon_ntff_profile_hook(h):
            mod._hook = h

        mod.get_axon_ntff_profile_hook = get_axon_ntff_profile_hook
        mod.set_axon_ntff_profile_hook = set_axon_ntff_profile_hook
        sys.modules["antenv.axon_hooks"] = mod
        antenv.axon_hooks = mod
    except Exception as e:  # pragma: no cover - profiling is best-effort
        print(f"ntff hook install failed: {e}")


def kernel(**inputs):
    global LAST_RESULT
    import os
    inputs = {k: np.asarray(v) for k, v in inputs.items()}
    if "nc" not in _cache:
        _cache["nc"] = _build()
    nc = _cache["nc"]

    shared = _prep_shared(inputs)
    x = inputs["x"].astype(np.float32)
    enc = inputs["encoder_output"].astype(np.float32)

    in_maps = []
    for core in range(NCORES):
        sl = slice(core * BL, (core + 1) * BL)
        xT = np.ascontiguousarray(x[sl].transpose(2, 0, 1)).reshape(C, TB)
        eT = np.ascontiguousarray(enc[sl].transpose(2, 0, 1)).reshape(C, TB)
        m = dict(shared)
        m["xT"] = xT
        m["xTb"] = xT.astype(_BF)
        m["eT"] = eT.astype(_BF)
        in_maps.append(m)

    trace = bool(int(os.environ.get("KERNEL_TRACE", "0")))
    if trace:
        _install_ntff_hook()
    from concourse.bass_utils import run_bass_kernel_spmd
    res = run_bass_kernel_spmd(nc, in_maps, list(range(NCORES)), trace=trace,
                               trace_cores=[0])
    LAST_RESULT = res

    out = np.empty((B, T, C), np.float32)
    for core in range(NCORES):
        outT = res.results[core]["outT"]  # [C, TB]
        out[core * BL:(core + 1) * BL] = outT.reshape(C, BL, T).transpose(1, 2, 0)
    return out
